# revision 14
# baseline (speedup 1.0000x reference)
"""Trainium2 Bass kernel: Chebyshev graph convolution.

Computes  out = sum_k A_k @ (x @ W_k) + bias  where A_k are sparse COO
matrices sharing one sparsity pattern (edge_row/edge_col) with per-degree
values adj_vals[k].

Restructured as:
    G      = x[edge_col]                       (gather, once per edge)
    Y_k    = segment_sum(adj_vals[k] * G)      (grouped one-hot matmuls on PE)
    out    = sum_k Y_k @ W_k + bias            (dense, fused into the scatter)

Sharding: destination-node range across 8 cores (6250 rows each), x
replicated in each core's HBM, zero cross-core communication.

Device pipeline per "unit" (512 sorted+padded edges = 32 groups of 16
edges, each group targeting a single dest row; j = 32*cc + 8*k + g):
    G[e, (u,cc,fi)]  = x[col[e], :]             (ONE indirect DMA per
                                                 16-unit round, bf16; x is
                                                 stored flat so the DMA is
                                                 costed at full row width)
    P4w[e, (u,j)]    = pattern[e,g] * a_k[e]    (one DVE mul per round)
    P2[(u,j), d]     = (iota_d == skey[u,j])    (one DVE op per round)
    Yp[fi, (s,j)]   += G_cc^T @ P4w_cc          (M1, PSUM batched 8 units)
    ZpT[fo, (s,j)]   = W_k^T @ Yp[:, kslice]    (M2, PSUM batched)
    Zp               = ZpT^T                    (PE transpose per unit)
    acc[d, fo]      += P2^T @ Zp                (M3, accumulated over the
                                                 block's units in PSUM f32)
    out[block]       = acc + bias               (DVE add, DMA out)

All PE/DVE traffic is bf16 (PSUM accumulation stays f32); output f32.
"""

import numpy as np

N_NODES = 50000
N_EDGES = 800000
F = 128
K = 4
N_CORES = 8
RPC = N_NODES // N_CORES      # rows per core
NBLK = (RPC + 127) // 128     # dest blocks per core (49)
GS = 16                       # edges per group (one dest row per group)
GPU = 32                      # groups per unit
EPU = GS * GPU                # edges per unit (512)
R = 16                        # units per staging round (one gather/round)
CB = 8                        # units per PSUM copy batch (divides R)

F32 = np.float32
I32 = np.int32


# ----------------------------------------------------------------------------
# Host-side preprocessing: shard + sort + pad the edge list, build payloads.
# ----------------------------------------------------------------------------

def _preprocess(adj_vals, edge_row, edge_col):
    """Build per-core payload arrays and the shared static schedule.

    Returns (U_bs, payloads) where U_bs is the per-block unit count (shared
    by all cores) and payloads[c] = dict(colidx, a4, skey) round-padded.
    """
    er = np.asarray(edge_row, dtype=np.int64)
    ec = np.asarray(edge_col, dtype=np.int64)
    adj = np.asarray(adj_vals, dtype=F32)            # [K, E]

    core = er // RPC
    per_core = []
    gpb_all = np.zeros((N_CORES, NBLK), dtype=np.int64)
    for c in range(N_CORES):
        sel = np.nonzero(core == c)[0]
        rloc = (er[sel] - c * RPC).astype(np.int64)
        order = np.argsort(rloc, kind="stable")
        eidx = sel[order]
        rs = rloc[order]
        counts = np.bincount(rs, minlength=RPC)       # edges per dest row
        gpr = -(-counts // GS)                        # groups per row
        gpr_pad = np.zeros(NBLK * 128, dtype=np.int64)
        gpr_pad[:RPC] = gpr
        gpb_all[c] = gpr_pad.reshape(NBLK, 128).sum(1)
        per_core.append((eidx, rs, counts, gpr))

    U_bs = np.maximum((-(-gpb_all // GPU)).max(axis=0), 1)  # units per block
    U = int(U_bs.sum())
    NR = -(-U // R)                                   # staging rounds

    blk_grp_base = np.concatenate([[0], np.cumsum(U_bs * GPU)])[:-1]  # [NBLK]

    payloads = []
    for c in range(N_CORES):
        eidx, rs, counts, gpr = per_core[c]
        # group base per row: groups of earlier rows in the same block,
        # offset by the block's group base.
        gpr_cum = np.concatenate([[0], np.cumsum(gpr)])  # [RPC+1]
        row_block = np.arange(RPC) // 128
        blk_first_row = row_block * 128
        grp_base_row = (blk_grp_base[row_block]
                        + gpr_cum[:RPC] - gpr_cum[blk_first_row])
        slot_base_row = grp_base_row * GS

        # scatter real edges into padded slots
        row_start = np.concatenate([[0], np.cumsum(counts)])  # [RPC+1]
        within = np.arange(len(rs)) - row_start[rs]
        pos = slot_base_row[rs] + within

        nslot = U * EPU
        cols_flat = np.zeros(nslot, dtype=I32)
        a4_flat = np.zeros((nslot, K), dtype=F32)
        cols_flat[pos] = ec[eidx].astype(I32)
        a4_flat[pos] = adj[:, eidx].T

        # dest slot per group
        ngrp = U * GPU
        skey_grp = np.zeros(ngrp, dtype=F32)
        totg = int(gpr.sum())
        row_ids = np.repeat(np.arange(RPC), gpr)
        grp_within = np.arange(totg) - np.repeat(gpr_cum[:RPC], gpr)
        grp_pos = np.repeat(grp_base_row, gpr) + grp_within
        skey_grp[grp_pos] = (row_ids % 128).astype(F32)

        # device layouts
        colidx = cols_flat.reshape(U, 4, 128).transpose(0, 2, 1)       # [U,128,4]
        a4 = (a4_flat.reshape(U, 4, 128, K).transpose(0, 2, 1, 3)
              .reshape(U, 128, 4 * K))                                 # [U,128,16]
        skey = np.tile(skey_grp.reshape(U, GPU), (1, 4))               # [U,128]

        # round-padded staging layouts
        Upad = NR * R
        colidx_p = np.zeros((Upad, 128, 4), dtype=I32)
        a4_p = np.zeros((Upad, 128, 16), dtype=F32)
        skey_p = np.zeros((Upad, 128), dtype=F32)
        colidx_p[:U], a4_p[:U], skey_p[:U] = colidx, a4, skey
        payloads.append(dict(
            colidx=np.ascontiguousarray(
                colidx_p.reshape(NR, R, 128, 4).transpose(0, 2, 1, 3)
                .reshape(NR, 128, R * 4)),
            a4=np.ascontiguousarray(
                a4_p.reshape(NR, R, 128, 16).transpose(0, 2, 1, 3)
                .reshape(NR, 128, R * 16)),
            skey=np.ascontiguousarray(
                skey_p.reshape(NR, R, 128).transpose(0, 2, 1)),
        ))
    return U_bs.astype(np.int64), payloads


def _np_dt(use_bf16):
    if use_bf16:
        import ml_dtypes
        return ml_dtypes.bfloat16
    return F32


def _host_constants(weight, bias, use_bf16=True):
    cdt = _np_dt(use_bf16)
    w_packed = np.ascontiguousarray(
        np.asarray(weight, dtype=F32).transpose(1, 0, 2)
        .reshape(F, K * F).astype(cdt))
    bias_rep = np.broadcast_to(
        np.asarray(bias, dtype=F32)[None, :], (128, F)).copy()
    # pattern tiled over the R units of a round: col (u, cc, g, k) -> p//GS==g
    # (g sits mid-column so the a4 broadcast keeps a packed innermost k dim,
    # which enables the DVE 2x bf16 mode)
    p = np.arange(128)
    g_of_col = np.tile((np.arange(128) % 32) // K, R)
    pattern_r = (p[:, None] // GS == g_of_col[None, :]).astype(cdt)  # [128,R*128]
    # iota tiled over units: col (u, d) -> d
    iota_r = np.tile(np.arange(128, dtype=F32)[None, :], (128, R)).astype(cdt)
    return w_packed, bias_rep, pattern_r, iota_r


# ----------------------------------------------------------------------------
# Numpy emulation of the device pipeline (layout validation / debugging).
# ----------------------------------------------------------------------------

def _emulate(x, weight, bias, U_bs, payloads):
    x = np.asarray(x, dtype=F32)
    w_packed, bias_rep, pattern_r, iota_r = _host_constants(
        weight, bias, use_bf16=False)
    pattern = pattern_r[:, :128]
    iota = iota_r[:, :128]
    out = np.zeros((N_NODES, F), dtype=F32)
    for c in range(N_CORES):
        pl = payloads[c]
        NR = pl["colidx"].shape[0]
        colidx = (pl["colidx"].reshape(NR, 128, R, 4).transpose(0, 2, 1, 3)
                  .reshape(NR * R, 128, 4))
        a4 = (pl["a4"].reshape(NR, 128, R, 16).transpose(0, 2, 1, 3)
              .reshape(NR * R, 128, 16))
        skey = pl["skey"].transpose(0, 2, 1).reshape(NR * R, 128)
        u = 0
        for b in range(NBLK):
            acc = np.zeros((128, F), dtype=F32)
            for _ in range(int(U_bs[b])):
                # p4w columns (c, g, k)
                p4w = pattern * a4[u].reshape(128, 4, 1, 4).repeat(
                    8, axis=2).reshape(128, 128)
                yp = np.zeros((F, 128), dtype=F32)
                for cc in range(4):
                    G = x[colidx[u, :, cc]]                    # [128e, F]
                    yp[:, 32 * cc:32 * cc + 32] = (
                        G.T @ p4w[:, 32 * cc:32 * cc + 32])
                zT = np.zeros((F, 128), dtype=F32)
                for k in range(K):
                    cols = (32 * np.arange(4)[:, None] * 1
                            + 4 * np.arange(8)[None, :] + k).ravel()
                    zT[:, 32 * k:32 * k + 32] = (
                        w_packed[:, F * k:F * (k + 1)].T @ yp[:, cols])
                zp = zT.T                                      # [128j, F]
                p2 = (iota == skey[u][:, None]).astype(F32)    # [128j, 128d]
                acc += p2.T @ zp
                u += 1
            rows = min(128, RPC - 128 * b)
            out[c * RPC + 128 * b: c * RPC + 128 * b + rows] = (
                acc[:rows] + bias_rep[:rows])
    return out


# ----------------------------------------------------------------------------
# Bass kernel builder.
# ----------------------------------------------------------------------------

def _build(U_bs, NR, use_bf16=True):
    import concourse.bacc as bacc
    import concourse.bass as bass
    import concourse.mybir as mybir
    import concourse.tile as tile
    from concourse.masks import make_identity

    f32 = mybir.dt.float32
    i32 = mybir.dt.int32
    cdt = mybir.dt.bfloat16 if use_bf16 else f32

    nc = bacc.Bacc("TRN2", target_bir_lowering=False, debug=False,
                   enable_asserts=False, num_devices=N_CORES)

    # x stored FLAT so the indirect gather's source AP is fully contiguous:
    # the DMA is then costed at the (wide) destination-row granularity.
    # Indices are premultiplied by F on the host.
    x_d = nc.dram_tensor("x", [1, N_NODES * F], cdt, kind="ExternalInput")
    w_d = nc.dram_tensor("w", [F, K * F], cdt, kind="ExternalInput")
    bias_d = nc.dram_tensor("bias_rep", [128, F], f32, kind="ExternalInput")
    pattern_d = nc.dram_tensor("pattern_r", [128, R * 128], cdt,
                               kind="ExternalInput")
    iota_d = nc.dram_tensor("iota_r", [128, R * 128], cdt,
                            kind="ExternalInput")
    colidx_d = nc.dram_tensor("colidx", [NR, 128, R * 4], i32,
                              kind="ExternalInput")
    # a4 (R*16 cols) and skey (R cols) staged together in one bf16 tensor so
    # the per-partition transfer is >= 512B (full DMA descriptor rate).
    askey_d = nc.dram_tensor("askey", [NR, 128, R * 17], cdt,
                             kind="ExternalInput")
    out_d = nc.dram_tensor("out", [RPC, F], f32, kind="ExternalOutput")

    # unit -> (block, unit-in-block, units-in-block)
    units = []
    for b in range(NBLK):
        nu = int(U_bs[b])
        for ul in range(nu):
            units.append((b, ul, nu))
    U = len(units)

    with tile.TileContext(nc) as tc:
        with (
            tc.tile_pool(name="const", bufs=1) as constp,
            tc.tile_pool(name="stage", bufs=3) as stagep,
            tc.tile_pool(name="gpool", bufs=3) as gpool,
            tc.tile_pool(name="rwork", bufs=3) as rwork,
            tc.tile_pool(name="cbuf", bufs=3) as cbuf,
            tc.tile_pool(name="psum", bufs=1, space="PSUM") as psump,
            tc.tile_pool(name="accp", bufs=2, space="PSUM") as accp,
            tc.tile_pool(name="outp", bufs=2) as outp,
        ):
            w_sb = constp.tile([F, K * F], cdt)
            nc.sync.dma_start(out=w_sb[:], in_=w_d[:])
            bias_sb = constp.tile([128, F], f32)
            nc.sync.dma_start(out=bias_sb[:], in_=bias_d[:])
            pattern_sb = constp.tile([128, R * 128], cdt)
            nc.sync.dma_start(out=pattern_sb[:], in_=pattern_d[:])
            iota_sb = constp.tile([128, R * 128], cdt)
            nc.sync.dma_start(out=iota_sb[:], in_=iota_d[:])
            identity = constp.tile([128, 128], cdt)
            make_identity(nc, identity[:])

            col_t = askey_t = g_t = p4w_t = p2_t = None
            acc_ps = None
            for u0 in range(0, U, CB):
                nb = min(CB, U - u0)
                rnd, ur0 = divmod(u0, R)
                if ur0 == 0:
                    # ---- stage one round: payload DMAs + batched gather +
                    # batched DVE precomputes for all R units -----------------
                    col_t = stagep.tile([128, R * 4], i32, tag="col")
                    nc.sync.dma_start(out=col_t[:], in_=colidx_d[rnd])
                    askey_t = stagep.tile([128, R * 17], cdt, tag="askey")
                    nc.sync.dma_start(out=askey_t[:], in_=askey_d[rnd])

                    g_t = gpool.tile([128, R * 4 * F], cdt, tag="g")
                    nc.gpsimd.indirect_dma_start(
                        out=g_t[:], out_offset=None,
                        in_=x_d[:],
                        in_offset=bass.IndirectOffsetOnAxis(
                            ap=col_t[:], axis=1))

                    # P4w[(u,cc,g,k)] = pattern * a4 (broadcast over mid g;
                    # innermost k stays packed -> DVE 2x bf16 mode)
                    p4w_t = rwork.tile([128, R * 128], cdt, tag="p4w")
                    nc.vector.tensor_tensor(
                        out=p4w_t[:].rearrange(
                            "p (uc g k) -> p uc g k", uc=4 * R, k=K),
                        in0=pattern_sb[:].rearrange(
                            "p (uc g k) -> p uc g k", uc=4 * R, k=K),
                        in1=(askey_t[:, :R * 16]
                             .rearrange("p (uc k) -> p uc () k", k=K)
                             .broadcast_to([128, 4 * R, 8, K])),
                        op=mybir.AluOpType.mult)

                    # P2[(u,j), d] = (iota_d == skey[u,j])
                    p2_t = rwork.tile([128, R * 128], cdt, tag="p2")
                    nc.vector.tensor_tensor(
                        out=p2_t[:].rearrange("p (u d) -> p u d", u=R),
                        in0=iota_sb[:].rearrange("p (u d) -> p u d", u=R),
                        in1=askey_t[:, R * 16:R * 17]
                            .broadcast_to([128, R, 128]),
                        op=mybir.AluOpType.is_equal)

                # ---- M1: Yp[fi, (s,j)] = sum_cc G_cc^T @ P4w_cc ------------
                yp_ps = psump.tile([128, CB * 128], mybir.dt.float32,
                                   tag="yp")
                for s in range(nb):
                    ur = ur0 + s
                    for cc in range(4):
                        nc.tensor.matmul(
                            out=yp_ps[:, s * 128 + 32 * cc:
                                      s * 128 + 32 * cc + 32],
                            lhsT=g_t[:, (ur * 4 + cc) * F:
                                     (ur * 4 + cc + 1) * F],
                            rhs=p4w_t[:, ur * 128 + 32 * cc:
                                      ur * 128 + 32 * cc + 32],
                            start=(cc == 0), stop=(cc == 3))
                yp_sb = cbuf.tile([128, CB * 128], cdt, tag="ypsb")
                nc.scalar.copy(out=yp_sb[:, :nb * 128],
                               in_=yp_ps[:, :nb * 128])

                # ---- M2: ZpT[fo, (s,kslice)] = W_k^T @ Yp[:, kslice] -------
                z_ps = psump.tile([128, CB * 128], mybir.dt.float32, tag="z")
                yp_r = yp_sb[:].rearrange(
                    "p (s c g k) -> p s c g k", s=CB, c=4, k=K)
                for s in range(nb):
                    for k in range(K):
                        nc.tensor.matmul(
                            out=z_ps[:, s * 128 + 32 * k:
                                     s * 128 + 32 * k + 32],
                            lhsT=w_sb[:, F * k:F * (k + 1)],
                            rhs=yp_r[:, s, :, :, k],
                            start=(k == 0), stop=(k == 3))
                zT_sb = cbuf.tile([128, CB * 128], cdt, tag="ztsb")
                nc.scalar.copy(out=zT_sb[:, :nb * 128],
                               in_=z_ps[:, :nb * 128])

                # ---- transpose ZpT -> Zp (per unit) ------------------------
                zp_ps = psump.tile([128, CB * 128], cdt, tag="zp")
                for s in range(nb):
                    nc.tensor.transpose(
                        out=zp_ps[:, s * 128:(s + 1) * 128],
                        in_=zT_sb[:, s * 128:(s + 1) * 128],
                        identity=identity[:])
                zp_sb = cbuf.tile([128, CB * 128], cdt, tag="zpsb")
                nc.vector.tensor_copy(out=zp_sb[:, :nb * 128],
                                      in_=zp_ps[:, :nb * 128])

                # ---- M3: acc[d, fo] += P2^T @ Zp (per unit, PSUM f32) ------
                for s in range(nb):
                    b, ul, nu = units[u0 + s]
                    if ul == 0:
                        acc_ps = accp.tile([128, F], mybir.dt.float32,
                                           tag="acc")
                    nc.tensor.matmul(
                        out=acc_ps[:],
                        lhsT=p2_t[:, (ur0 + s) * 128:(ur0 + s + 1) * 128],
                        rhs=zp_sb[:, s * 128:(s + 1) * 128],
                        start=(ul == 0), stop=(ul == nu - 1))
                    if ul == nu - 1:
                        out_sb = outp.tile([128, F], f32, tag="out")
                        nc.vector.tensor_tensor(
                            out=out_sb[:], in0=acc_ps[:], in1=bias_sb[:],
                            op=mybir.AluOpType.add)
                        rows = min(128, RPC - 128 * b)
                        nc.sync.dma_start(
                            out=out_d[128 * b:128 * b + rows, :],
                            in_=out_sb[:rows, :])
    nc.compile()
    return nc


def _make_in_maps(x, weight, bias, payloads, use_bf16=True):
    w_packed, bias_rep, pattern_r, iota_r = _host_constants(
        weight, bias, use_bf16)
    cdt = _np_dt(use_bf16)
    x = np.ascontiguousarray(
        np.asarray(x, dtype=F32).astype(cdt).reshape(1, -1))
    in_maps = []
    for c in range(N_CORES):
        pl = payloads[c]
        askey = np.concatenate(
            [pl["a4"].astype(F32), pl["skey"]], axis=2).astype(cdt)
        in_maps.append({
            "x": x, "w": w_packed, "bias_rep": bias_rep,
            "pattern_r": pattern_r, "iota_r": iota_r,
            "colidx": np.ascontiguousarray(pl["colidx"] * F),
            "askey": np.ascontiguousarray(askey),
        })
    return in_maps


USE_BF16 = True


def _prepare(x, weight, bias, adj_vals, edge_row, edge_col, use_bf16=None):
    if use_bf16 is None:
        use_bf16 = USE_BF16
    U_bs, payloads = _preprocess(adj_vals, edge_row, edge_col)
    NR = payloads[0]["colidx"].shape[0]
    nc = _build(U_bs, NR, use_bf16)
    in_maps = _make_in_maps(x, weight, bias, payloads, use_bf16)
    return nc, in_maps


def kernel(x, weight, bias, adj_vals, edge_row, edge_col):
    from concourse.bass_utils import run_bass_kernel_spmd
    nc, in_maps = _prepare(x, weight, bias, adj_vals, edge_row, edge_col)
    res = run_bass_kernel_spmd(nc, in_maps, core_ids=list(range(N_CORES)))
    out = np.concatenate(
        [np.asarray(res.results[c]["out"]) for c in range(N_CORES)], axis=0)
    return out.astype(np.float32)


# revision 18
# speedup vs baseline: 1.0352x; 1.0352x over previous
"""Trainium2 Bass kernel: Chebyshev graph convolution.

Computes  out = sum_k A_k @ (x @ W_k) + bias  where A_k are sparse COO
matrices sharing one sparsity pattern (edge_row/edge_col) with per-degree
values adj_vals[k].

Restructured as:
    G      = x[edge_col]                       (gather, once per edge)
    Y_k    = segment_sum(adj_vals[k] * G)      (grouped one-hot matmuls on PE)
    out    = sum_k Y_k @ W_k + bias            (dense, fused into the scatter)

Sharding: destination-node range across 8 cores (6250 rows each), x
replicated in each core's HBM, zero cross-core communication.

Device pipeline per "unit" (512 sorted+padded edges = 32 groups of 16
edges, each group targeting a single dest row; j = 32*cc + 8*k + g):
    G[e, (u,cc,fi)]  = x[col[e], :]             (ONE indirect DMA per
                                                 16-unit round, bf16; x is
                                                 stored flat so the DMA is
                                                 costed at full row width)
    P4w[e, (u,j)]    = pattern[e,g] * a_k[e]    (one DVE mul per round)
    P2[(u,j), d]     = (iota_d == skey[u,j])    (one DVE op per round)
    Yp[fi, (s,j)]   += G_cc^T @ P4w_cc          (M1, PSUM batched 8 units)
    ZpT[fo, (s,j)]   = W_k^T @ Yp[:, kslice]    (M2, PSUM batched)
    Zp               = ZpT^T                    (PE transpose per unit)
    acc[d, fo]      += P2^T @ Zp                (M3, accumulated over the
                                                 block's units in PSUM f32)
    out[block]       = acc + bias               (DVE add, DMA out)

All PE/DVE traffic is bf16 (PSUM accumulation stays f32); output f32.
"""

import numpy as np

N_NODES = 50000
N_EDGES = 800000
F = 128
K = 4
N_CORES = 8
RPC = N_NODES // N_CORES      # rows per core
NBLK = (RPC + 127) // 128     # dest blocks per core (49)
GS = 16                       # edges per group (one dest row per group)
GPU = 32                      # groups per unit
EPU = GS * GPU                # edges per unit (512)
R = 16                        # units per staging round (one gather/round)
CB = 8                        # units per PSUM copy batch (divides R)

F32 = np.float32
I32 = np.int32


# ----------------------------------------------------------------------------
# Host-side preprocessing: shard + sort + pad the edge list, build payloads.
# ----------------------------------------------------------------------------

def _preprocess(adj_vals, edge_row, edge_col):
    """Build per-core payload arrays and the shared static schedule.

    Returns (U_bs, payloads) where U_bs is the per-block unit count (shared
    by all cores) and payloads[c] = dict(colidx, a4, skey) round-padded.
    """
    er = np.asarray(edge_row, dtype=np.int64)
    ec = np.asarray(edge_col, dtype=np.int64)
    adj = np.asarray(adj_vals, dtype=F32)            # [K, E]

    core = er // RPC
    per_core = []
    gpb_all = np.zeros((N_CORES, NBLK), dtype=np.int64)
    for c in range(N_CORES):
        sel = np.nonzero(core == c)[0]
        rloc = (er[sel] - c * RPC).astype(np.int64)
        order = np.argsort(rloc, kind="stable")
        eidx = sel[order]
        rs = rloc[order]
        counts = np.bincount(rs, minlength=RPC)       # edges per dest row
        gpr = -(-counts // GS)                        # groups per row
        gpr_pad = np.zeros(NBLK * 128, dtype=np.int64)
        gpr_pad[:RPC] = gpr
        gpb_all[c] = gpr_pad.reshape(NBLK, 128).sum(1)
        per_core.append((eidx, rs, counts, gpr))

    U_bs = np.maximum((-(-gpb_all // GPU)).max(axis=0), 1)  # units per block
    U = int(U_bs.sum())
    NR = -(-U // R)                                   # staging rounds

    blk_grp_base = np.concatenate([[0], np.cumsum(U_bs * GPU)])[:-1]  # [NBLK]

    payloads = []
    for c in range(N_CORES):
        eidx, rs, counts, gpr = per_core[c]
        # group base per row: groups of earlier rows in the same block,
        # offset by the block's group base.
        gpr_cum = np.concatenate([[0], np.cumsum(gpr)])  # [RPC+1]
        row_block = np.arange(RPC) // 128
        blk_first_row = row_block * 128
        grp_base_row = (blk_grp_base[row_block]
                        + gpr_cum[:RPC] - gpr_cum[blk_first_row])
        slot_base_row = grp_base_row * GS

        # scatter real edges into padded slots
        row_start = np.concatenate([[0], np.cumsum(counts)])  # [RPC+1]
        within = np.arange(len(rs)) - row_start[rs]
        pos = slot_base_row[rs] + within

        nslot = U * EPU
        cols_flat = np.zeros(nslot, dtype=I32)
        a4_flat = np.zeros((nslot, K), dtype=F32)
        cols_flat[pos] = ec[eidx].astype(I32)
        a4_flat[pos] = adj[:, eidx].T

        # dest slot per group
        ngrp = U * GPU
        skey_grp = np.zeros(ngrp, dtype=F32)
        totg = int(gpr.sum())
        row_ids = np.repeat(np.arange(RPC), gpr)
        grp_within = np.arange(totg) - np.repeat(gpr_cum[:RPC], gpr)
        grp_pos = np.repeat(grp_base_row, gpr) + grp_within
        skey_grp[grp_pos] = (row_ids % 128).astype(F32)

        # device layouts
        colidx = cols_flat.reshape(U, 4, 128).transpose(0, 2, 1)       # [U,128,4]
        a4 = (a4_flat.reshape(U, 4, 128, K).transpose(0, 2, 1, 3)
              .reshape(U, 128, 4 * K))                                 # [U,128,16]
        skey = np.tile(skey_grp.reshape(U, GPU), (1, 4))               # [U,128]

        # round-padded staging layouts
        Upad = NR * R
        colidx_p = np.zeros((Upad, 128, 4), dtype=I32)
        a4_p = np.zeros((Upad, 128, 16), dtype=F32)
        skey_p = np.zeros((Upad, 128), dtype=F32)
        colidx_p[:U], a4_p[:U], skey_p[:U] = colidx, a4, skey
        payloads.append(dict(
            colidx=np.ascontiguousarray(
                colidx_p.reshape(NR, R, 128, 4).transpose(0, 2, 1, 3)
                .reshape(NR, 128, R * 4)),
            a4=np.ascontiguousarray(
                a4_p.reshape(NR, R, 128, 16).transpose(0, 2, 1, 3)
                .reshape(NR, 128, R * 16)),
            skey=np.ascontiguousarray(
                skey_p.reshape(NR, R, 128).transpose(0, 2, 1)),
        ))
    return U_bs.astype(np.int64), payloads


def _np_dt(use_bf16):
    if use_bf16:
        import ml_dtypes
        return ml_dtypes.bfloat16
    return F32


def _host_constants(weight, bias, use_bf16=True):
    cdt = _np_dt(use_bf16)
    w_packed = np.ascontiguousarray(
        np.asarray(weight, dtype=F32).transpose(1, 0, 2)
        .reshape(F, K * F).astype(cdt))
    bias_rep = np.broadcast_to(
        np.asarray(bias, dtype=F32)[None, :], (128, F)).copy()
    # pattern tiled over the R units of a round: col (u, cc, g, k) -> p//GS==g
    # (g sits mid-column so the a4 broadcast keeps a packed innermost k dim,
    # which enables the DVE 2x bf16 mode)
    p = np.arange(128)
    g_of_col = np.tile((np.arange(128) % 32) // K, R)
    pattern_r = (p[:, None] // GS == g_of_col[None, :]).astype(cdt)  # [128,R*128]
    # iota: col d -> d (broadcast over units on-device)
    iota_r = np.tile(np.arange(128, dtype=F32)[None, :], (128, 1)).astype(cdt)
    return w_packed, bias_rep, pattern_r, iota_r


# ----------------------------------------------------------------------------
# Numpy emulation of the device pipeline (layout validation / debugging).
# ----------------------------------------------------------------------------

def _emulate(x, weight, bias, U_bs, payloads):
    x = np.asarray(x, dtype=F32)
    w_packed, bias_rep, pattern_r, iota_r = _host_constants(
        weight, bias, use_bf16=False)
    pattern = pattern_r[:, :128]
    iota = iota_r[:, :128]
    out = np.zeros((N_NODES, F), dtype=F32)
    for c in range(N_CORES):
        pl = payloads[c]
        NR = pl["colidx"].shape[0]
        colidx = (pl["colidx"].reshape(NR, 128, R, 4).transpose(0, 2, 1, 3)
                  .reshape(NR * R, 128, 4))
        a4 = (pl["a4"].reshape(NR, 128, R, 16).transpose(0, 2, 1, 3)
              .reshape(NR * R, 128, 16))
        skey = pl["skey"].transpose(0, 2, 1).reshape(NR * R, 128)
        u = 0
        for b in range(NBLK):
            acc = np.zeros((128, F), dtype=F32)
            for _ in range(int(U_bs[b])):
                # p4w columns (c, g, k)
                p4w = pattern * a4[u].reshape(128, 4, 1, 4).repeat(
                    8, axis=2).reshape(128, 128)
                yp = np.zeros((F, 128), dtype=F32)
                for cc in range(4):
                    G = x[colidx[u, :, cc]]                    # [128e, F]
                    yp[:, 32 * cc:32 * cc + 32] = (
                        G.T @ p4w[:, 32 * cc:32 * cc + 32])
                zT = np.zeros((F, 128), dtype=F32)
                for k in range(K):
                    cols = (32 * np.arange(4)[:, None] * 1
                            + 4 * np.arange(8)[None, :] + k).ravel()
                    zT[:, 32 * k:32 * k + 32] = (
                        w_packed[:, F * k:F * (k + 1)].T @ yp[:, cols])
                zp = zT.T                                      # [128j, F]
                p2 = (iota == skey[u][:, None]).astype(F32)    # [128j, 128d]
                acc += p2.T @ zp
                u += 1
            rows = min(128, RPC - 128 * b)
            out[c * RPC + 128 * b: c * RPC + 128 * b + rows] = (
                acc[:rows] + bias_rep[:rows])
    return out


# ----------------------------------------------------------------------------
# Bass kernel builder.
# ----------------------------------------------------------------------------

def _build(U_bs, NR, use_bf16=True):
    import concourse.bacc as bacc
    import concourse.bass as bass
    import concourse.mybir as mybir
    import concourse.tile as tile
    from concourse.masks import make_identity

    f32 = mybir.dt.float32
    i32 = mybir.dt.int32
    cdt = mybir.dt.bfloat16 if use_bf16 else f32

    nc = bacc.Bacc("TRN2", target_bir_lowering=False, debug=False,
                   enable_asserts=False, num_devices=N_CORES)

    # x stored FLAT so the indirect gather's source AP is fully contiguous:
    # the DMA is then costed at the (wide) destination-row granularity.
    # Indices are premultiplied by F on the host.
    x_d = nc.dram_tensor("x", [1, N_NODES * F], cdt, kind="ExternalInput")
    w_d = nc.dram_tensor("w", [F, K * F], cdt, kind="ExternalInput")
    bias_d = nc.dram_tensor("bias_rep", [128, F], f32, kind="ExternalInput")
    pattern_d = nc.dram_tensor("pattern_r", [128, R * 128], cdt,
                               kind="ExternalInput")
    iota_d = nc.dram_tensor("iota_r", [128, 128], cdt,
                            kind="ExternalInput")
    colidx_d = nc.dram_tensor("colidx", [NR, 128, R * 4], i32,
                              kind="ExternalInput")
    # a4 (R*16 cols) and skey (R cols) staged together in one bf16 tensor so
    # the per-partition transfer is >= 512B (full DMA descriptor rate).
    askey_d = nc.dram_tensor("askey", [NR, 128, R * 17], cdt,
                             kind="ExternalInput")
    out_d = nc.dram_tensor("out", [RPC, F], f32, kind="ExternalOutput")

    # unit -> (block, unit-in-block, units-in-block)
    units = []
    for b in range(NBLK):
        nu = int(U_bs[b])
        for ul in range(nu):
            units.append((b, ul, nu))
    U = len(units)

    with tile.TileContext(nc) as tc:
        with (
            tc.tile_pool(name="const", bufs=1) as constp,
            tc.tile_pool(name="stage", bufs=3) as stagep,
            tc.tile_pool(name="gpool", bufs=3) as gpool,
            tc.tile_pool(name="rwork", bufs=3) as rwork,
            tc.tile_pool(name="cbuf", bufs=3) as cbuf,
            tc.tile_pool(name="psum", bufs=1, space="PSUM") as psump,
            tc.tile_pool(name="accp", bufs=2, space="PSUM") as accp,
            tc.tile_pool(name="outp", bufs=2) as outp,
        ):
            def stage_round(rnd):
                col_t = stagep.tile([128, R * 4], i32, tag="col")
                nc.sync.dma_start(out=col_t[:], in_=colidx_d[rnd])
                askey_t = stagep.tile([128, R * 17], cdt, tag="askey")
                nc.sync.dma_start(out=askey_t[:], in_=askey_d[rnd])
                g_t = gpool.tile([128, R * 4 * F], cdt, tag="g")
                nc.gpsimd.indirect_dma_start(
                    out=g_t[:], out_offset=None,
                    in_=x_d[:],
                    in_offset=bass.IndirectOffsetOnAxis(
                        ap=col_t[:], axis=1))
                return col_t, askey_t, g_t

            # round-0 payload + gather go FIRST so the serial DMA chain
            # starts immediately; constants load in its shadow.
            col_t, askey_t, g_t = stage_round(0)

            w_sb = constp.tile([F, K * F], cdt)
            nc.sync.dma_start(out=w_sb[:], in_=w_d[:])
            bias_sb = constp.tile([128, F], f32)
            nc.sync.dma_start(out=bias_sb[:], in_=bias_d[:])
            pattern_sb = constp.tile([128, R * 128], cdt)
            nc.sync.dma_start(out=pattern_sb[:], in_=pattern_d[:])
            iota_sb = constp.tile([128, 128], cdt)
            nc.sync.dma_start(out=iota_sb[:], in_=iota_d[:])
            identity = constp.tile([128, 128], cdt)
            make_identity(nc, identity[:])

            p4w_t = p2_t = None
            acc_ps = None
            for u0 in range(0, U, CB):
                nb = min(CB, U - u0)
                rnd, ur0 = divmod(u0, R)
                if ur0 == 0:
                    if rnd > 0:
                        col_t, askey_t, g_t = stage_round(rnd)

                    # P4w[(u,cc,g,k)] = pattern * a4 (broadcast over mid g;
                    # innermost k stays packed -> DVE 2x bf16 mode)
                    p4w_t = rwork.tile([128, R * 128], cdt, tag="p4w")
                    nc.vector.tensor_tensor(
                        out=p4w_t[:].rearrange(
                            "p (uc g k) -> p uc g k", uc=4 * R, k=K),
                        in0=pattern_sb[:].rearrange(
                            "p (uc g k) -> p uc g k", uc=4 * R, k=K),
                        in1=(askey_t[:, :R * 16]
                             .rearrange("p (uc k) -> p uc () k", k=K)
                             .broadcast_to([128, 4 * R, 8, K])),
                        op=mybir.AluOpType.mult)

                    # P2[(u,j), d] = (iota_d == skey[u,j])
                    p2_t = rwork.tile([128, R * 128], cdt, tag="p2")
                    nc.vector.tensor_tensor(
                        out=p2_t[:].rearrange("p (u d) -> p u d", u=R),
                        in0=iota_sb[:].rearrange("p d -> p () d")
                            .broadcast_to([128, R, 128]),
                        in1=askey_t[:, R * 16:R * 17]
                            .broadcast_to([128, R, 128]),
                        op=mybir.AluOpType.is_equal)

                # ---- M1: Yp[fi, (s,j)] = sum_cc G_cc^T @ P4w_cc ------------
                yp_ps = psump.tile([128, CB * 128], mybir.dt.float32,
                                   tag="yp")
                for s in range(nb):
                    ur = ur0 + s
                    for cc in range(4):
                        nc.tensor.matmul(
                            out=yp_ps[:, s * 128 + 32 * cc:
                                      s * 128 + 32 * cc + 32],
                            lhsT=g_t[:, (ur * 4 + cc) * F:
                                     (ur * 4 + cc + 1) * F],
                            rhs=p4w_t[:, ur * 128 + 32 * cc:
                                      ur * 128 + 32 * cc + 32],
                            start=(cc == 0), stop=(cc == 3))
                yp_sb = cbuf.tile([128, CB * 128], cdt, tag="ypsb")
                nc.scalar.copy(out=yp_sb[:, :nb * 128],
                               in_=yp_ps[:, :nb * 128])

                # ---- M2: ZpT[fo, (s,kslice)] = W_k^T @ Yp[:, kslice] -------
                z_ps = psump.tile([128, CB * 128], mybir.dt.float32, tag="z")
                yp_r = yp_sb[:].rearrange(
                    "p (s c g k) -> p s c g k", s=CB, c=4, k=K)
                for s in range(nb):
                    for k in range(K):
                        nc.tensor.matmul(
                            out=z_ps[:, s * 128 + 32 * k:
                                     s * 128 + 32 * k + 32],
                            lhsT=w_sb[:, F * k:F * (k + 1)],
                            rhs=yp_r[:, s, :, :, k],
                            start=(k == 0), stop=(k == 3))
                zT_sb = cbuf.tile([128, CB * 128], cdt, tag="ztsb")
                nc.scalar.copy(out=zT_sb[:, :nb * 128],
                               in_=z_ps[:, :nb * 128])

                # ---- transpose ZpT -> Zp (per unit) ------------------------
                zp_ps = psump.tile([128, CB * 128], cdt, tag="zp")
                for s in range(nb):
                    nc.tensor.transpose(
                        out=zp_ps[:, s * 128:(s + 1) * 128],
                        in_=zT_sb[:, s * 128:(s + 1) * 128],
                        identity=identity[:])
                zp_sb = cbuf.tile([128, CB * 128], cdt, tag="zpsb")
                nc.vector.tensor_copy(out=zp_sb[:, :nb * 128],
                                      in_=zp_ps[:, :nb * 128])

                # ---- M3: acc[d, fo] += P2^T @ Zp (per unit, PSUM f32) ------
                for s in range(nb):
                    b, ul, nu = units[u0 + s]
                    if ul == 0:
                        acc_ps = accp.tile([128, F], mybir.dt.float32,
                                           tag="acc")
                    nc.tensor.matmul(
                        out=acc_ps[:],
                        lhsT=p2_t[:, (ur0 + s) * 128:(ur0 + s + 1) * 128],
                        rhs=zp_sb[:, s * 128:(s + 1) * 128],
                        start=(ul == 0), stop=(ul == nu - 1))
                    if ul == nu - 1:
                        out_sb = outp.tile([128, F], f32, tag="out")
                        nc.vector.tensor_tensor(
                            out=out_sb[:], in0=acc_ps[:], in1=bias_sb[:],
                            op=mybir.AluOpType.add)
                        rows = min(128, RPC - 128 * b)
                        nc.sync.dma_start(
                            out=out_d[128 * b:128 * b + rows, :],
                            in_=out_sb[:rows, :])
    nc.compile()
    return nc


def _make_in_maps(x, weight, bias, payloads, use_bf16=True):
    w_packed, bias_rep, pattern_r, iota_r = _host_constants(
        weight, bias, use_bf16)
    cdt = _np_dt(use_bf16)
    x = np.ascontiguousarray(
        np.asarray(x, dtype=F32).astype(cdt).reshape(1, -1))
    in_maps = []
    for c in range(N_CORES):
        pl = payloads[c]
        askey = np.concatenate(
            [pl["a4"].astype(F32), pl["skey"]], axis=2).astype(cdt)
        in_maps.append({
            "x": x, "w": w_packed, "bias_rep": bias_rep,
            "pattern_r": pattern_r, "iota_r": iota_r,
            "colidx": np.ascontiguousarray(pl["colidx"] * F),
            "askey": np.ascontiguousarray(askey),
        })
    return in_maps


USE_BF16 = True


def _prepare(x, weight, bias, adj_vals, edge_row, edge_col, use_bf16=None):
    if use_bf16 is None:
        use_bf16 = USE_BF16
    U_bs, payloads = _preprocess(adj_vals, edge_row, edge_col)
    NR = payloads[0]["colidx"].shape[0]
    nc = _build(U_bs, NR, use_bf16)
    in_maps = _make_in_maps(x, weight, bias, payloads, use_bf16)
    return nc, in_maps


def kernel(x, weight, bias, adj_vals, edge_row, edge_col):
    from concourse.bass_utils import run_bass_kernel_spmd
    nc, in_maps = _prepare(x, weight, bias, adj_vals, edge_row, edge_col)
    res = run_bass_kernel_spmd(nc, in_maps, core_ids=list(range(N_CORES)))
    out = np.concatenate(
        [np.asarray(res.results[c]["out"]) for c in range(N_CORES)], axis=0)
    return out.astype(np.float32)
